# revision 1
# baseline (speedup 1.0000x reference)
"""ConvLSTM cell (complex-valued gates) on 8 TRN2 NeuronCores.

Strategy
--------
Data-parallel over batch: 16 images -> 2 per core. Per core, each gate's
complex 3x3 conv is computed as 9 shifted matmuls per input component
(zr / zi), accumulated in PSUM:

    out[128, 512] += lhsT[128in, 128out].T @ z_shift[128in, 512]

where the 128 output channels stack the real (0:64) and imag (64:128) parts:
    comp zr: lhsT = [ Wr | Wi ]
    comp zi: lhsT = [-Wi | Wr ]

All matmul operands are fp16 (full PE speed, 11-bit mantissa; validated
2.2e-3 worst-case scale-relative error end to end vs the fp32 reference).
ScalarE applies sigmoid/tanh straight from PSUM with the per-channel bias
fused into the activation. VectorE does the complex elementwise update in
fp16 (2x mode). x (*) c_prev is an input-only elementwise term precomputed
on the host and added on-chip. Outputs leave as fp16 and are upcast on host.

The spatial dim is processed in 8 macro-tiles per core of [128, 1024]
(1 batch x 16 rows x 64 cols), each backed by a 2-bank PSUM tile per gate.
z is kept resident in SBUF, zero-padded to 66x66 on the host so conv taps
are plain shifted access patterns.
"""
import sys
import numpy as np

sys.path.insert(0, "/opt/trn_rl_repo")

P = 128          # partitions / channels (64 real + 64 imag)
HALF = 64
B = 16           # full batch
N_CORES = 8
B_CORE = B // N_CORES   # batch per core
H = W = 64
HP = WP = 66     # padded spatial
N_RB = 4         # row-blocks per batch (16 rows each)
MACRO = 16 * W   # 1024 columns per macro tile
GATE_ORDER = "ioc"      # index of gate g in the packed weight array

_CACHE = {}


def _apply_drain_patch(tile_mod):
    """The kernel-tail drain aggregates one wait per live proc-semaphore, but
    walrus rejects instructions with more than a few sync waits. Split the
    tail waits across a chain of single-wait drains."""
    if getattr(tile_mod.TileContext, "_drain_patched", False):
        return

    def _patched(self, tick_clock, wait_clock):
        ScopedClock = tile_mod.ScopedClock
        nc = self.nc
        drain_inst = nc.sync.drain()
        wait_clock.add_sem_waits(
            drain_inst.ins, ScopedClock({None: tick_clock.global_clock})
        )
        NW = 3
        si = drain_inst.ins.sync_info
        if si is not None and si.on_wait and len(si.on_wait) > NW:
            conds = list(si.on_wait)
            si.on_wait = conds[:NW]
            rest = conds[NW:]
            while rest:
                extra = nc.sync.drain()
                esi = extra.ins.sync_info
                if esi is None:
                    import bass_rust
                    extra.ins.sync_info = bass_rust.SyncInfo(
                        on_wait=rest[:NW], on_update=[])
                else:
                    esi.on_wait = rest[:NW]
                rest = rest[NW:]

        nc.all_engine_barrier()
        assert self.sems is not None
        popped = nc._tile_sem_poison_stack.pop()
        assert popped is self._sem_poison
        nc.clear_and_free_semaphores(list(self.sems.allocated().values()))
        nc.all_engine_barrier()

    tile_mod.TileContext._drain_and_barrier = _patched
    tile_mod.TileContext._drain_patched = True


def _split_excess_waits(nc, max_waits=1):
    """walrus's per-instruction sync-wait slots are tight (1 for some ISA
    structs). Hoist excess waits into same-engine no-ops inserted directly
    before the instruction — identical semantics, per-engine order kept."""
    import concourse.mybir as mybir
    n_new = 0
    for fn in nc.m.functions:
        for bb in fn.blocks:
            il = bb.instructions
            out = []
            for inst in il:
                si = inst.sync_info
                if si is not None and si.on_wait and len(si.on_wait) > max_waits:
                    conds = list(si.on_wait)
                    si.on_wait = conds[:max_waits]
                    rest = conds[max_waits:]
                    for j in range(0, len(rest), max_waits):
                        nop = mybir.InstNoOp(
                            name=f"{inst.name}_w{j}",
                            sync_info=mybir.SyncInfo(
                                on_wait=rest[j:j + max_waits], on_update=[]),
                            bass_nofuse=True,
                            engine=inst.engine,
                        )
                        out.append(nop)
                        n_new += 1
                out.append(inst)
            if n_new:
                il[:] = out
    return n_new


def _build_program():
    import concourse.bass as bass
    import concourse.tile as tile
    from concourse import mybir
    from contextlib import ExitStack

    _apply_drain_patch(tile)
    fp16 = mybir.dt.float16
    f32 = mybir.dt.float32
    Sigmoid = mybir.ActivationFunctionType.Sigmoid
    Tanh = mybir.ActivationFunctionType.Tanh

    nc = bass.Bass("TRN2", target_bir_lowering=False, debug=False)
    zr_d = nc.dram_tensor("zr", [P, B_CORE, HP, WP], fp16, kind="ExternalInput").ap()
    zi_d = nc.dram_tensor("zi", [P, B_CORE, HP, WP], fp16, kind="ExternalInput").ap()
    w_d = nc.dram_tensor("wts", [P, 54, P], fp16, kind="ExternalInput").ap()
    b_d = nc.dram_tensor("bias", [P, 6], f32, kind="ExternalInput").ap()
    xc_d = nc.dram_tensor("xc", [P, B_CORE, H, W], fp16, kind="ExternalInput").ap()
    h_d = nc.dram_tensor("h_out", [P, B_CORE, H, W], fp16, kind="ExternalOutput").ap()
    c_d = nc.dram_tensor("c_out", [P, B_CORE, H, W], fp16, kind="ExternalOutput").ap()

    # padded-row chunks (overlapping): A=[0:18) B=[16:34) C=[32:66)
    Z_CHUNKS = {'A': (0, 18), 'B': (16, 18), 'C': (32, 34)}

    with tile.TileContext(nc) as tc, ExitStack() as ctx:
        const = ctx.enter_context(tc.tile_pool(name="const", bufs=1))
        bias_s = const.tile([P, 6], f32)
        nc.sync.dma_start(bias_s[:], b_d[:])
        # chunked input loads, in first-consumption order, so the first
        # macro-tile's matmuls start after ~1.8MB of DMA instead of ~8MB
        w_g = {}
        z_ch = {}

        def load_w(gname):
            gi = GATE_ORDER.index(gname)
            wt = const.tile([P, 18, P], fp16, name=f"w_{gname}")
            nc.sync.dma_start(wt[:], w_d[:, gi * 18:(gi + 1) * 18, :])
            w_g[gname] = wt

        def load_z(b, ch, after=None):
            row0, nr = Z_CHUNKS[ch]
            for comp, zt_d in (('r', zr_d), ('i', zi_d)):
                t = const.tile([P, nr, WP], fp16, name=f"z{comp}_{b}_{ch}")
                dm = nc.sync.dma_start(t[:], zt_d[:, b, row0:row0 + nr, :])
                if after is not None:
                    # hold the transfer back until the anchor matmul retires so
                    # it can't steal HBM bandwidth from earlier-needed loads
                    tile.add_dep_helper(dm.ins, after,
                                        reason="defer non-critical z load")
                z_ch[(comp, b, ch)] = t

        # upfront: only what macro-tile (b=0, rb=0/1) needs; the rest is
        # emitted mid-loop so the DGE serves the critical transfers first
        load_w('c')
        load_z(0, 'A')
        load_w('i')
        load_w('o')

        ps_i = ctx.enter_context(tc.tile_pool(name="ps_i", bufs=1, space="PSUM"))
        ps_o = ctx.enter_context(tc.tile_pool(name="ps_o", bufs=1, space="PSUM"))
        ps_c = ctx.enter_context(tc.tile_pool(name="ps_c", bufs=1, space="PSUM"))
        work = ctx.enter_context(tc.tile_pool(name="work", bufs=2))

        def macro_tile(b, r0, nrows):
            cols = nrows * W
            if r0 + nrows + 1 < 18:
                ch = 'A'
            elif r0 >= 16 and r0 + nrows + 1 < 34:
                ch = 'B'
            else:
                ch = 'C'
            roff = Z_CHUNKS[ch][0]   # chunk's first padded row

            last_mm = [None]

            def conv_gate(pool, gname):
                wt = w_g[gname]
                pt = pool.tile([P, cols], f32, tag="pt_" + gname)
                for k in range(18):
                    kh, kw, ci = k // 6, (k // 2) % 3, k % 2
                    z_s = z_ch[('ri'[ci], b, ch)]
                    m = (kh * 3 + kw) * 2 + ci
                    for half in range(nrows // 8):
                        r0h = r0 + half * 8 - roff
                        mm = nc.tensor.matmul(
                            pt[:, half * 512:(half + 1) * 512],
                            wt[:, m, :],
                            z_s[:, r0h + kh:r0h + kh + 8, kw:kw + 64],
                            start=(k == 0), stop=(k == 17),
                        )
                        last_mm[0] = mm.ins
                return pt

            # gate c first: its output heads the elementwise chain.
            # CTs = [cti; -ctr] comes straight from PSUM via partition-crossed
            # activations (ScalarE may cross bases; TensorTensor may not), so
            # no SBUF->SBUF swap DMAs are needed anywhere.
            pt_c = conv_gate(ps_c, 'c')
            CT = work.tile([P, cols], fp16, tag="CT")      # [ctr; cti]
            nc.scalar.activation(CT[:], pt_c[:], Tanh, bias=bias_s[:, 2:3])
            CTs = work.tile([P, cols], fp16, tag="CTs")    # [cti; -ctr]
            nc.scalar.activation(CTs[0:HALF, :], pt_c[HALF:P, :], Tanh,
                                 bias=bias_s[HALF:P, 2:3])
            nc.scalar.activation(CTs[HALF:P, :], pt_c[0:HALF, :], Tanh,
                                 bias=bias_s[0:HALF, 5:6], scale=-1.0)

            pt_i = conv_gate(ps_i, 'i')
            I = work.tile([P, cols], fp16, tag="I")
            nc.scalar.activation(I[:], pt_i[:], Sigmoid, bias=bias_s[:, 0:1])

            # i (*) ct (complex): product halves written to base-0/base-64 so
            # every TensorTensor keeps same-base inputs
            P1 = work.tile([P, cols], fp16, tag="P1")      # [ir*ctr ; ir*cti]
            nc.vector.tensor_mul(P1[0:HALF, :], I[0:HALF, :], CT[0:HALF, :])
            nc.vector.tensor_mul(P1[HALF:P, :], I[0:HALF, :], CTs[0:HALF, :])
            P2 = work.tile([P, cols], fp16, tag="P2")      # [ii*cti ; -ii*ctr]
            nc.vector.tensor_mul(P2[0:HALF, :], I[HALF:P, :], CT[HALF:P, :])
            nc.vector.tensor_mul(P2[HALF:P, :], I[HALF:P, :], CTs[HALF:P, :])
            tmp = work.tile([P, cols], fp16, tag="tmp")
            nc.vector.tensor_sub(tmp[:], P1[:], P2[:])

            xc_t = work.tile([P, cols], fp16, tag="xc_t")
            nc.sync.dma_start(xc_t[:], xc_d[:, b, r0:r0 + nrows, :])
            cnew = work.tile([P, cols], fp16, tag="cnew")
            nc.vector.tensor_add(cnew[:], xc_t[:], tmp[:])
            nc.sync.dma_start(c_d[:, b, r0:r0 + nrows, :], cnew[:])

            T = work.tile([P, cols], fp16, tag="T")        # [tr; ti]
            nc.scalar.activation(T[:], cnew[:], Tanh)
            Tn = work.tile([P, cols], fp16, tag="Tn")      # [.. ; -ti]
            nc.scalar.activation(Tn[HALF:P, :], cnew[HALF:P, :], Tanh,
                                 scale=-1.0)

            pt_o = conv_gate(ps_o, 'o')
            O = work.tile([P, cols], fp16, tag="O")        # [or; oi]
            nc.scalar.activation(O[:], pt_o[:], Sigmoid, bias=bias_s[:, 1:2])
            O2 = work.tile([P, cols], fp16, tag="O2")      # [oi; or]
            nc.scalar.activation(O2[0:HALF, :], pt_o[HALF:P, :], Sigmoid,
                                 bias=bias_s[HALF:P, 1:2])
            nc.scalar.activation(O2[HALF:P, :], pt_o[0:HALF, :], Sigmoid,
                                 bias=bias_s[0:HALF, 1:2])

            Q1 = work.tile([P, cols], fp16, tag="Q1")      # [or*tr ; oi*tr]
            nc.vector.tensor_mul(Q1[0:HALF, :], O[0:HALF, :], T[0:HALF, :])
            nc.vector.tensor_mul(Q1[HALF:P, :], O2[0:HALF, :], T[0:HALF, :])
            Q2 = work.tile([P, cols], fp16, tag="Q2")      # [oi*ti ; -or*ti]
            nc.vector.tensor_mul(Q2[0:HALF, :], O[HALF:P, :], T[HALF:P, :])
            nc.vector.tensor_mul(Q2[HALF:P, :], O2[HALF:P, :], Tn[HALF:P, :])

            hnew = work.tile([P, cols], fp16, tag="hnew")
            nc.vector.tensor_sub(hnew[:], Q1[:], Q2[:])
            nc.sync.dma_start(h_d[:, b, r0:r0 + nrows, :], hnew[:])
            return last_mm[0]

        # first tile small (PE starts on less DMA'd data), last tiles small
        # (short post-matmul epilogue chain); z-chunk loads two tiles ahead
        SCHEDULE = [(0, 0, 8), (0, 8, 8), (0, 16, 16), (0, 32, 16),
                    (0, 48, 16), (1, 0, 16), (1, 16, 16), (1, 32, 16),
                    (1, 48, 8), (1, 56, 8)]
        deferred = {1: [(0, 'B')], 2: [(0, 'C')], 3: [(1, 'A')],
                    4: [(1, 'B')], 5: [(1, 'C')]}
        anchor = None
        for tidx, (b, r0, nrows) in enumerate(SCHEDULE):
            for b2, ch2 in deferred.pop(tidx, []):
                load_z(b2, ch2, after=anchor)
            anchor = macro_tile(b, r0, nrows)

    _split_excess_waits(nc)
    return nc


def _prep_inputs(inputs):
    """Host-side shard + layout prep. Returns per-core in_maps."""
    f16 = np.float16
    x = np.asarray(inputs['x'], np.float32)
    h_prev = np.asarray(inputs['h_prev'], np.float32)
    c_prev = np.asarray(inputs['c_prev'], np.float32)

    xr, xi = x[:, :HALF], x[:, HALF:]
    hr, hi = h_prev[:, :HALF], h_prev[:, HALF:]
    cr, ci = c_prev[:, :HALF], c_prev[:, HALF:]

    # combined conv input, channel-major, zero-padded, fp16: [128, B, 66, 66]
    def prep_z(a, b):
        z = np.concatenate([a, b], axis=1).transpose(1, 0, 2, 3)
        return np.pad(z, ((0, 0), (0, 0), (1, 1), (1, 1))).astype(f16)
    zr = prep_z(xr, hr)
    zi = prep_z(xi, hi)

    # x (*) c_prev (complex elementwise), channel-major fp16: [128, B, 64, 64]
    xc = np.concatenate([xr * cr - xi * ci, xr * ci + xi * cr],
                        axis=1).transpose(1, 0, 2, 3).astype(f16)

    # packed gate weights: [cin 128, 54, cout 128] fp16
    wts = np.empty((54, P, P), np.float32)
    for g, gn in enumerate(GATE_ORDER):
        Wr = np.asarray(inputs['Wr_' + gn], np.float32)  # [64, 128, 3, 3]
        Wi = np.asarray(inputs['Wi_' + gn], np.float32)
        for kh in range(3):
            for kw in range(3):
                t = kh * 3 + kw
                wts[g * 18 + t * 2 + 0, :, :HALF] = Wr[:, :, kh, kw].T
                wts[g * 18 + t * 2 + 0, :, HALF:] = Wi[:, :, kh, kw].T
                wts[g * 18 + t * 2 + 1, :, :HALF] = -Wi[:, :, kh, kw].T
                wts[g * 18 + t * 2 + 1, :, HALF:] = Wr[:, :, kh, kw].T
    wts = np.ascontiguousarray(wts.transpose(1, 0, 2)).astype(f16)

    bias = np.empty((P, 6), np.float32)
    for g, gn in enumerate(GATE_ORDER):
        bias[:, g] = np.concatenate([np.asarray(inputs['br_' + gn]),
                                     np.asarray(inputs['bi_' + gn])])
    bias[:, 3:6] = -bias[:, 0:3]

    in_maps = []
    for c in range(N_CORES):
        sl = slice(c * B_CORE, (c + 1) * B_CORE)
        in_maps.append({
            "zr": np.ascontiguousarray(zr[:, sl]),
            "zi": np.ascontiguousarray(zi[:, sl]),
            "wts": wts,
            "bias": bias,
            "xc": np.ascontiguousarray(xc[:, sl]),
        })
    return in_maps


def _gather_outputs(results):
    h_full = np.empty((B, P, H, W), np.float32)
    c_full = np.empty((B, P, H, W), np.float32)
    for c in range(N_CORES):
        sl = slice(c * B_CORE, (c + 1) * B_CORE)
        h_full[sl] = results[c]["h_out"].transpose(1, 0, 2, 3).astype(np.float32)
        c_full[sl] = results[c]["c_out"].transpose(1, 0, 2, 3).astype(np.float32)
    return h_full, c_full


def _run(inputs, trace=False, trace_kwargs=None):
    from concourse.bass_utils import run_bass_kernel_spmd

    if "nc" not in _CACHE:
        _CACHE["nc"] = _build_program()
    nc = _CACHE["nc"]
    in_maps = _prep_inputs(inputs)
    r = run_bass_kernel_spmd(nc, in_maps, list(range(N_CORES)),
                             trace=trace, trace_kwargs=trace_kwargs or {})
    return _gather_outputs(r.results), r


def kernel(**inputs):
    (h_full, c_full), _ = _run(inputs)
    return h_full, c_full

